# revision 41
# baseline (speedup 1.0000x reference)
"""IF spiking-neuron scan (charge / fire / hard-reset) on 8 Trainium2 cores.

Reference recurrence over t (elementwise on every [B, N] element):
    u_t = v_{t-1} + x_t          # charge
    s_t = (u_t >= 1.0)           # fire
    v_t = (1 - s_t) * u_t        # hard reset to 0

Sharding: pure data parallel over the B*N = 262144 element chains; each
of the 8 cores owns 32768 chains as a [128 partitions, 256 cols] tile
per timestep, with zero communication.

Kernel design (68.4 us -> 48.1 us on HW):

* The recurrence is rewritten on the pre-reset potential u:
      u_{t+1} = (u_t if u_t < 1 else 0) + x_{t+1}
  which is ONE fused DVE instruction per timestep via a custom DVE op
  registered at import (documented extension point, concourse.dve_ops):
      IF_STEP_ANT: out = select(Src0 < C0, Src0, Zero) + Src1
  This halves vector-engine work vs the classic add + cmp/mult pair
  (measured 425 ns vs 2x205 ns per timestep) and materializes u_t in
  SBUF for the spike extraction. The whole scan must stay on the DVE:
  the Pool engine's core-v3 firmware only implements Add/Multiply
  tensor_tensor ops (no compares, no TensorScalarPtr), and the scalar
  engine is unary.

* Spikes on the scalar engine, off the critical path, one op per block:
      r = Sign(1.0 - u) -> uint8
  r == 1 iff u < 1 (no spike); u >= 1 gives 0 or the uint8 cast of
  -1.0 (0 saturating / 255 wrapping - both decode the same), so the
  host computes s = (r != 1). Sign(0) = 0 keeps u == V_TH ties exact.
  The second-to-last block's Sign runs in two chunks so most of it
  overlaps the scan; the final 1-timestep block's spikes run on the DVE
  (is_lt, same polarity) so the drain skips the ACT round-trip.

* DMA choreography (the subtle part - the DGE generates descriptors at
  ~2.2 ns each, serialized per ring in trigger order, and hands them to
  the 16 ~22 GB/s queues in chunks, so both descriptor size and trigger
  placement matter):
  - Ramp (first 10 timesteps): streamed from a host-interleaved
    [RAMP_T*2, P, 128] tensor whose 512 B descriptors spread every
    timestep across queues - the first block lands ~0.9 us after its
    trigger and the scan starts at ~11 us instead of ~14.
  - Steady state (t >= 10): streamed from the flat [T, P, F] tensor
    with 1 KiB descriptors; at 512 B the generator's ~228 GB/s cannot
    feed the DVE's ~278 GB/s demand, at 1 KiB it can (~457 GB/s).
  - Outputs ([P, T, F] uint8, per-partition-contiguous descriptors)
    ride the scalar engine's ring so input triggers never queue behind
    them; the Pool ring measured ~4 us slower for the same transfers.
  All fp32 arithmetic is bit-identical to the reference (one add and
  one compare per element-step, in reference order).
"""

import numpy as np

import concourse.tile as tile
from concourse import bacc, mybir
from concourse.bass_utils import run_bass_kernel_spmd

T = 64
B = 32
N = 8192
NCORES = 8
PERCORE = (B * N) // NCORES  # 32768 element chains per core
P = 128                      # SBUF partitions
F = PERCORE // P             # 256 elements per partition
F2 = 2                       # input interleave factor (descriptor spread)
F1 = F // F2

V_TH = 1.0

# Timestep blocks. The first RAMP_T timesteps stream from a host-
# interleaved [RAMP_T*2, P, F1] tensor (512 B descriptors, each timestep
# split over two DMA queues) so the first block lands fast; the rest
# stream from the flat [T, P, F] tensor (1 KiB descriptors) because the
# DGE generates descriptors at only ~2.2 ns each: at 512 B that's
# ~228 GB/s of supply against the DVE's ~278 GB/s demand and the scan
# starves, at 1 KiB supply is ~457 GB/s. The last block is a single
# timestep whose spikes run on the DVE (is_lt) to minimize the drain.
RAMP_BLOCKS = [4, 6]
MAIN_BLOCKS = [8] * 6 + [5, 1]
RAMP_T = sum(RAMP_BLOCKS)
assert RAMP_T + sum(MAIN_BLOCKS) == T

_NC_CACHE = {}
_OP_CACHE = {}


def _register_if_step_op():
    """Register the fused IF-neuron step as a custom DVE op.

    Uses the documented extension point (concourse.dve_ops.OPS): the op
    body lowers to a single steady-state uop program whose sha is pinned
    at registration, the sub-opcode row is taken from the free range
    [1, 0x20), and the numpy reference makes CoreSim scheduling exact.
    (A hand-built 2X_2PORT dual-chain variant was tried and is NOT kept:
    the engine never engages the 2x slots for fp32 operands, so the op
    runs at the regular ~1.37 cycles/elem either way.)
    """
    if "op" in _OP_CACHE:
        return _OP_CACHE["op"]

    import concourse.dve_ops as dve_ops
    from concourse.dve_spec import Spec, Src0, Src1, C0, Zero, select, lower, _has_src1
    from concourse.dve_uop import DveOpSpec

    name = "IF_STEP_ANT"

    def _ref(in0, in1, c0, c1, c2):
        u = np.where(
            in0.astype(np.float32) < np.float32(c0),
            in0.astype(np.float32),
            np.float32(0.0),
        ).astype(np.float32)
        return (u + in1.astype(np.float32)).astype(np.float32)

    spec = Spec(body=select(Src0 < C0, Src0, Zero) + Src1, reference=_ref)

    existing = {op.name: op for op in dve_ops.OPS}
    if name in existing:
        _OP_CACHE["op"] = existing[name]
        return existing[name]

    row = 1 + len(dve_ops.OPS)
    shas = {}
    for ver in ("v3", "v4"):
        try:
            uops = lower(spec, ver=ver)
            shas[ver] = DveOpSpec(
                name=name, opcode=row, uops=uops, rd1_en=_has_src1(spec)
            ).sha(ver)
        except Exception:
            pass  # ver not supported in this build; TRN2 only needs v3

    op = dve_ops.DveOp(name, spec, subdim=False, uops_sha=shas)
    dve_ops.OPS.append(op)
    dve_ops._SUB_OPCODE_FOR_NAME[name] = row
    dve_ops.CUSTOM_DVE_SPECS[name] = spec
    _OP_CACHE["op"] = op
    return op


def build_nc():
    if_step = _register_if_step_op()
    # Bacc (not raw Bass): its compile() splits multi-wait sync conditions
    # into nop/event-semaphore prefixes — walrus accepts at most one sync
    # wait per hardware instruction.
    nc = bacc.Bacc("TRN2", target_bir_lowering=False, debug=False)
    # Ramp input: first RAMP_T timesteps, host-interleaved [RAMP_T*2, P, F1]
    xr_t = nc.dram_tensor(
        "xr", [RAMP_T * F2, P, F1], mybir.dt.float32, kind="ExternalInput"
    ).ap()
    # Main input: the flat [T, P, F] tensor (only t >= RAMP_T is read)
    xm_t = nc.dram_tensor(
        "xm", [T, P, F], mybir.dt.float32, kind="ExternalInput"
    ).ap()
    # y per-partition contiguous: per (p, t) a 256 B run, so a block's
    # write is one tb*256 B descriptor per partition — descriptor
    # generation stays trivial on the output ring.
    y = nc.dram_tensor("y", [P, T, F], mybir.dt.uint8, kind="ExternalOutput").ap()

    # DRAM-side APs with the partition dim first (the AP pairing matches
    # the SBUF partition dim against the leading DRAM dim).
    xrd = xr_t.rearrange("t2 p f1 -> p t2 f1")
    xmd = xm_t.rearrange("t p f -> p t f")

    with tile.TileContext(nc) as tc:
        with (
            tc.tile_pool(name="ub", bufs=4) as upool,
            tc.tile_pool(name="sout", bufs=4) as spool,
        ):
            prev = None  # tile whose prev_lo F-slice holds u_{t-1}
            prev_lo = 0
            t0 = 0
            blocks = [(tb, True) for tb in RAMP_BLOCKS]
            blocks += [(tb, False) for tb in MAIN_BLOCKS]
            for bi, (tb, ramp) in enumerate(blocks):
                last = bi == len(blocks) - 1
                # x streams straight into the u tile; each step rewrites
                # its own slice in place (u_t = f(u_{t-1}, x_t) with x_t
                # read from and u_t written to the same address — safe
                # elementwise in-order). This removes the whole x pool
                # and its per-block buffer-reuse semaphores, and u_0 is
                # just x_0 (v_0 = 0), so t = 0 needs no instruction at
                # all: the scan is 63 ops and there is no zero tile.
                ub = upool.tile([P, tb * F], mybir.dt.float32, tag="ub")
                if ramp:
                    uv = ub[:].rearrange(
                        "p (t2 f1) -> p t2 f1", t2=F2 * tb, f1=F1
                    )
                    nc.sync.dma_start(uv, xrd[:, F2 * t0:F2 * (t0 + tb), :])
                else:
                    nc.sync.dma_start(ub[:], xmd[:, t0:t0 + tb, :])
                for ti in range(tb):
                    lo = ti * F
                    if bi == 0 and ti == 0:
                        prev, prev_lo = ub, 0
                        continue
                    nc.vector._custom_dve(
                        if_step,
                        out=ub[:, lo:lo + F],
                        in0=prev[:, prev_lo:prev_lo + F],
                        in1=ub[:, lo:lo + F],
                        s0=V_TH,
                    )
                    prev, prev_lo = ub, lo
                st = spool.tile([P, tb * F], mybir.dt.uint8, tag="sout")
                if last:
                    # tail: spikes on the DVE right after the final step
                    # (r = (u < V_TH), same polarity as the Sign path)
                    nc.vector.tensor_scalar(
                        st[:], ub[:], V_TH, None, mybir.AluOpType.is_lt
                    )
                    nc.scalar.dma_start(y[:, t0:t0 + tb, :], st[:])
                else:
                    # r = Sign(V_TH - u) as uint8: 1 <=> no spike; spike
                    # rows are 0 (u == V_TH) or the cast of -1.0. Host
                    # decodes s = (r != 1); Sign(0) = 0 keeps exact
                    # threshold ties correct. The second-to-last block's
                    # Sign runs in two chunks so most of it overlaps the
                    # scan and only a short chunk remains in the drain.
                    chunks = [4, tb - 4] if bi == len(blocks) - 2 else [tb]
                    c0 = 0
                    for ct in chunks:
                        nc.scalar.activation(
                            st[:, c0 * F:(c0 + ct) * F],
                            ub[:, c0 * F:(c0 + ct) * F],
                            mybir.ActivationFunctionType.Sign,
                            bias=V_TH, scale=-1.0,
                        )
                        c0 += ct
                    # outputs ride the scalar engine's HW-DGE ring so
                    # input triggers never queue behind them on the SP
                    # ring.
                    nc.scalar.dma_start(y[:, t0:t0 + tb, :], st[:])
                t0 += tb
    nc.compile()
    return nc


def _get_nc():
    if "nc" not in _NC_CACHE:
        _NC_CACHE["nc"] = build_nc()
    return _NC_CACHE["nc"]


def run_sharded(x_seq, trace=False, nc=None, **kwargs):
    if nc is None:
        nc = _get_nc()
    x2 = np.ascontiguousarray(np.asarray(x_seq, dtype=np.float32)).reshape(T, B * N)
    in_maps = []
    for c in range(NCORES):
        xc = x2[:, c * PERCORE:(c + 1) * PERCORE].reshape(T, P, F)
        # ramp tensor: [RAMP_T, P, F] -> [RAMP_T, F2, P, F1] -> interleaved
        xr_host = np.ascontiguousarray(
            xc[:RAMP_T]
            .reshape(RAMP_T, P, F2, F1)
            .transpose(0, 2, 1, 3)
            .reshape(RAMP_T * F2, P, F1)
        )
        in_maps.append({"xr": xr_host, "xm": np.ascontiguousarray(xc)})
    # A cold device occasionally reports NRT_EXEC_UNIT_UNRECOVERABLE on the
    # first execute and recovers on the next attempt; retry a couple times.
    for attempt in range(3):
        try:
            res = run_bass_kernel_spmd(
                nc, in_maps, list(range(NCORES)), trace=trace, **kwargs
            )
            break
        except Exception:  # jax.errors.JaxRuntimeError and friends
            if attempt == 2:
                raise
            import time
            time.sleep(2.0)
    out = np.empty((T, B * N), dtype=np.float32)
    for c in range(NCORES):
        yc = np.asarray(res.results[c]["y"])          # [P, T, F] uint8
        r = yc.transpose(1, 0, 2).reshape(T, PERCORE)
        # r == 1 <=> no spike (u < V_TH); 0 and 255 both mean spike
        out[:, c * PERCORE:(c + 1) * PERCORE] = (r != 1)
    return out.reshape(T, B, N), res


def kernel(x_seq):
    out, _ = run_sharded(x_seq)
    return out


# revision 42
# speedup vs baseline: 1.0196x; 1.0196x over previous
"""IF spiking-neuron scan (charge / fire / hard-reset) on 8 Trainium2 cores.

Reference recurrence over t (elementwise on every [B, N] element):
    u_t = v_{t-1} + x_t          # charge
    s_t = (u_t >= 1.0)           # fire
    v_t = (1 - s_t) * u_t        # hard reset to 0

Sharding: pure data parallel over the B*N = 262144 element chains; each
of the 8 cores owns 32768 chains as a [128 partitions, 256 cols] tile
per timestep, with zero communication.

Kernel design (68.4 us -> 48.1 us on HW):

* The recurrence is rewritten on the pre-reset potential u:
      u_{t+1} = (u_t if u_t < 1 else 0) + x_{t+1}
  which is ONE fused DVE instruction per timestep via a custom DVE op
  registered at import (documented extension point, concourse.dve_ops):
      IF_STEP_ANT: out = select(Src0 < C0, Src0, Zero) + Src1
  This halves vector-engine work vs the classic add + cmp/mult pair
  (measured 425 ns vs 2x205 ns per timestep) and materializes u_t in
  SBUF for the spike extraction. The whole scan must stay on the DVE:
  the Pool engine's core-v3 firmware only implements Add/Multiply
  tensor_tensor ops (no compares, no TensorScalarPtr), and the scalar
  engine is unary.

* Spikes on the scalar engine, off the critical path, one op per block:
      r = Sign(1.0 - u) -> uint8
  r == 1 iff u < 1 (no spike); u >= 1 gives 0 or the uint8 cast of
  -1.0 (0 saturating / 255 wrapping - both decode the same), so the
  host computes s = (r != 1). Sign(0) = 0 keeps u == V_TH ties exact.
  The second-to-last block's Sign runs in two chunks so most of it
  overlaps the scan; the final 1-timestep block's spikes run on the DVE
  (is_lt, same polarity) so the drain skips the ACT round-trip.

* DMA choreography (the subtle part - the DGE generates descriptors at
  ~2.2 ns each, serialized per ring in trigger order, and hands them to
  the 16 ~22 GB/s queues in chunks, so both descriptor size and trigger
  placement matter):
  - Ramp (first 10 timesteps): streamed from a host-interleaved
    [RAMP_T*2, P, 128] tensor whose 512 B descriptors spread every
    timestep across queues - the first block lands ~0.9 us after its
    trigger and the scan starts at ~11 us instead of ~14.
  - Steady state (t >= 10): streamed from the flat [T, P, F] tensor
    with 1 KiB descriptors; at 512 B the generator's ~228 GB/s cannot
    feed the DVE's ~278 GB/s demand, at 1 KiB it can (~457 GB/s).
  - Outputs ([P, T, F] uint8, per-partition-contiguous descriptors)
    ride the scalar engine's ring so input triggers never queue behind
    them; the Pool ring measured ~4 us slower for the same transfers.
  All fp32 arithmetic is bit-identical to the reference (one add and
  one compare per element-step, in reference order).
"""

import numpy as np

import concourse.tile as tile
from concourse import bacc, mybir
from concourse.bass_utils import run_bass_kernel_spmd

T = 64
B = 32
N = 8192
NCORES = 8
PERCORE = (B * N) // NCORES  # 32768 element chains per core
P = 128                      # SBUF partitions
F = PERCORE // P             # 256 elements per partition
F2 = 2                       # input interleave factor (descriptor spread)
F1 = F // F2

V_TH = 1.0

# Timestep blocks. The first RAMP_T timesteps stream from a host-
# interleaved [RAMP_T*2, P, F1] tensor (512 B descriptors, each timestep
# split over two DMA queues) so the first block lands fast; the rest
# stream from the flat [T, P, F] tensor (1 KiB descriptors) because the
# DGE generates descriptors at only ~2.2 ns each: at 512 B that's
# ~228 GB/s of supply against the DVE's ~278 GB/s demand and the scan
# starves, at 1 KiB supply is ~457 GB/s. The last block is a single
# timestep whose spikes run on the DVE (is_lt) to minimize the drain.
RAMP_BLOCKS = [4, 3, 3]
MAIN_BLOCKS = [8] * 6 + [5, 1]
RAMP_T = sum(RAMP_BLOCKS)
assert RAMP_T + sum(MAIN_BLOCKS) == T

_NC_CACHE = {}
_OP_CACHE = {}


def _register_if_step_op():
    """Register the fused IF-neuron step as a custom DVE op.

    Uses the documented extension point (concourse.dve_ops.OPS): the op
    body lowers to a single steady-state uop program whose sha is pinned
    at registration, the sub-opcode row is taken from the free range
    [1, 0x20), and the numpy reference makes CoreSim scheduling exact.
    (A hand-built 2X_2PORT dual-chain variant was tried and is NOT kept:
    the engine never engages the 2x slots for fp32 operands, so the op
    runs at the regular ~1.37 cycles/elem either way.)
    """
    if "op" in _OP_CACHE:
        return _OP_CACHE["op"]

    import concourse.dve_ops as dve_ops
    from concourse.dve_spec import Spec, Src0, Src1, C0, Zero, select, lower, _has_src1
    from concourse.dve_uop import DveOpSpec

    name = "IF_STEP_ANT"

    def _ref(in0, in1, c0, c1, c2):
        u = np.where(
            in0.astype(np.float32) < np.float32(c0),
            in0.astype(np.float32),
            np.float32(0.0),
        ).astype(np.float32)
        return (u + in1.astype(np.float32)).astype(np.float32)

    spec = Spec(body=select(Src0 < C0, Src0, Zero) + Src1, reference=_ref)

    existing = {op.name: op for op in dve_ops.OPS}
    if name in existing:
        _OP_CACHE["op"] = existing[name]
        return existing[name]

    row = 1 + len(dve_ops.OPS)
    shas = {}
    for ver in ("v3", "v4"):
        try:
            uops = lower(spec, ver=ver)
            shas[ver] = DveOpSpec(
                name=name, opcode=row, uops=uops, rd1_en=_has_src1(spec)
            ).sha(ver)
        except Exception:
            pass  # ver not supported in this build; TRN2 only needs v3

    op = dve_ops.DveOp(name, spec, subdim=False, uops_sha=shas)
    dve_ops.OPS.append(op)
    dve_ops._SUB_OPCODE_FOR_NAME[name] = row
    dve_ops.CUSTOM_DVE_SPECS[name] = spec
    _OP_CACHE["op"] = op
    return op


def build_nc():
    if_step = _register_if_step_op()
    # Bacc (not raw Bass): its compile() splits multi-wait sync conditions
    # into nop/event-semaphore prefixes — walrus accepts at most one sync
    # wait per hardware instruction.
    nc = bacc.Bacc("TRN2", target_bir_lowering=False, debug=False)
    # Ramp input: first RAMP_T timesteps, host-interleaved [RAMP_T*2, P, F1]
    xr_t = nc.dram_tensor(
        "xr", [RAMP_T * F2, P, F1], mybir.dt.float32, kind="ExternalInput"
    ).ap()
    # Main input: the flat [T, P, F] tensor (only t >= RAMP_T is read)
    xm_t = nc.dram_tensor(
        "xm", [T, P, F], mybir.dt.float32, kind="ExternalInput"
    ).ap()
    # y per-partition contiguous: per (p, t) a 256 B run, so a block's
    # write is one tb*256 B descriptor per partition — descriptor
    # generation stays trivial on the output ring.
    y = nc.dram_tensor("y", [P, T, F], mybir.dt.uint8, kind="ExternalOutput").ap()

    # DRAM-side APs with the partition dim first (the AP pairing matches
    # the SBUF partition dim against the leading DRAM dim).
    xrd = xr_t.rearrange("t2 p f1 -> p t2 f1")
    xmd = xm_t.rearrange("t p f -> p t f")

    with tile.TileContext(nc) as tc:
        with (
            tc.tile_pool(name="ub", bufs=4) as upool,
            tc.tile_pool(name="sout", bufs=4) as spool,
        ):
            prev = None  # tile whose prev_lo F-slice holds u_{t-1}
            prev_lo = 0
            t0 = 0
            blocks = [(tb, True) for tb in RAMP_BLOCKS]
            blocks += [(tb, False) for tb in MAIN_BLOCKS]
            for bi, (tb, ramp) in enumerate(blocks):
                last = bi == len(blocks) - 1
                # x streams straight into the u tile; each step rewrites
                # its own slice in place (u_t = f(u_{t-1}, x_t) with x_t
                # read from and u_t written to the same address — safe
                # elementwise in-order). This removes the whole x pool
                # and its per-block buffer-reuse semaphores, and u_0 is
                # just x_0 (v_0 = 0), so t = 0 needs no instruction at
                # all: the scan is 63 ops and there is no zero tile.
                ub = upool.tile([P, tb * F], mybir.dt.float32, tag="ub")
                if ramp:
                    uv = ub[:].rearrange(
                        "p (t2 f1) -> p t2 f1", t2=F2 * tb, f1=F1
                    )
                    nc.sync.dma_start(uv, xrd[:, F2 * t0:F2 * (t0 + tb), :])
                else:
                    nc.sync.dma_start(ub[:], xmd[:, t0:t0 + tb, :])
                for ti in range(tb):
                    lo = ti * F
                    if bi == 0 and ti == 0:
                        prev, prev_lo = ub, 0
                        continue
                    nc.vector._custom_dve(
                        if_step,
                        out=ub[:, lo:lo + F],
                        in0=prev[:, prev_lo:prev_lo + F],
                        in1=ub[:, lo:lo + F],
                        s0=V_TH,
                    )
                    prev, prev_lo = ub, lo
                st = spool.tile([P, tb * F], mybir.dt.uint8, tag="sout")
                if last:
                    # tail: spikes on the DVE right after the final step
                    # (r = (u < V_TH), same polarity as the Sign path)
                    nc.vector.tensor_scalar(
                        st[:], ub[:], V_TH, None, mybir.AluOpType.is_lt
                    )
                    nc.scalar.dma_start(y[:, t0:t0 + tb, :], st[:])
                else:
                    # r = Sign(V_TH - u) as uint8: 1 <=> no spike; spike
                    # rows are 0 (u == V_TH) or the cast of -1.0. Host
                    # decodes s = (r != 1); Sign(0) = 0 keeps exact
                    # threshold ties correct. The second-to-last block's
                    # Sign runs in two chunks so most of it overlaps the
                    # scan and only a short chunk remains in the drain.
                    chunks = [4, tb - 4] if bi == len(blocks) - 2 else [tb]
                    c0 = 0
                    for ct in chunks:
                        nc.scalar.activation(
                            st[:, c0 * F:(c0 + ct) * F],
                            ub[:, c0 * F:(c0 + ct) * F],
                            mybir.ActivationFunctionType.Sign,
                            bias=V_TH, scale=-1.0,
                        )
                        c0 += ct
                    # outputs ride the scalar engine's HW-DGE ring so
                    # input triggers never queue behind them on the SP
                    # ring.
                    nc.scalar.dma_start(y[:, t0:t0 + tb, :], st[:])
                t0 += tb
    nc.compile()
    return nc


def _get_nc():
    if "nc" not in _NC_CACHE:
        _NC_CACHE["nc"] = build_nc()
    return _NC_CACHE["nc"]


def run_sharded(x_seq, trace=False, nc=None, **kwargs):
    if nc is None:
        nc = _get_nc()
    x2 = np.ascontiguousarray(np.asarray(x_seq, dtype=np.float32)).reshape(T, B * N)
    in_maps = []
    for c in range(NCORES):
        xc = x2[:, c * PERCORE:(c + 1) * PERCORE].reshape(T, P, F)
        # ramp tensor: [RAMP_T, P, F] -> [RAMP_T, F2, P, F1] -> interleaved
        xr_host = np.ascontiguousarray(
            xc[:RAMP_T]
            .reshape(RAMP_T, P, F2, F1)
            .transpose(0, 2, 1, 3)
            .reshape(RAMP_T * F2, P, F1)
        )
        in_maps.append({"xr": xr_host, "xm": np.ascontiguousarray(xc)})
    # A cold device occasionally reports NRT_EXEC_UNIT_UNRECOVERABLE on the
    # first execute and recovers on the next attempt; retry a couple times.
    for attempt in range(3):
        try:
            res = run_bass_kernel_spmd(
                nc, in_maps, list(range(NCORES)), trace=trace, **kwargs
            )
            break
        except Exception:  # jax.errors.JaxRuntimeError and friends
            if attempt == 2:
                raise
            import time
            time.sleep(2.0)
    out = np.empty((T, B * N), dtype=np.float32)
    for c in range(NCORES):
        yc = np.asarray(res.results[c]["y"])          # [P, T, F] uint8
        r = yc.transpose(1, 0, 2).reshape(T, PERCORE)
        # r == 1 <=> no spike (u < V_TH); 0 and 255 both mean spike
        out[:, c * PERCORE:(c + 1) * PERCORE] = (r != 1)
    return out.reshape(T, B, N), res


def kernel(x_seq):
    out, _ = run_sharded(x_seq)
    return out
